# revision 8
# baseline (speedup 1.0000x reference)
"""Trainium2 Bass kernel for nn_CriticRDDPG (RDDPG critic forward).

Data-parallel over batch across 8 NeuronCores (16 rows/core). All activations
live in transposed "featT" layout [feature_partition, batch_free]; the GRU
scan runs with bf16 stationary recurrent weights (fast weight load) and the
moving h^T operand, producing gate pre-activations directly in a packed
[128, 12, 16] PSUM layout so gate math uses full partitions and no
transposes are needed anywhere.
"""

import os
import sys

sys.path.insert(0, "/opt/trn_rl_repo")

import numpy as np
import ml_dtypes

import concourse.bass as bass
import concourse.tile as tile
from concourse import bacc, mybir
from concourse.bass_utils import run_bass_kernel_spmd

F32 = mybir.dt.float32
F32R = mybir.dt.float32r
BF16 = mybir.dt.bfloat16
ALU = mybir.AluOpType
ACTF = mybir.ActivationFunctionType

B, T, A = 128, 512, 256
NCORES = 8
BL = B // NCORES  # 16 batch rows per core
TCH = 32  # time steps per chunk
NCH = int(os.environ.get("KERNEL_NCH", T // TCH))  # chunks (16 full)
TT = NCH * TCH  # total steps compiled
UG = 512  # U_GRU
NJG = 12  # 1536 / 128 gate-feature tiles
NKC = 4  # 512 / 128 hidden k-chunks

_CACHE = {}


def _bcast(ap, reps, axis_len):
    """0-stride broadcast of a [128, n] AP to [128, n, reps] (or [128, reps, n])."""
    return bass.AP(
        tensor=ap.tensor,
        offset=ap.offset,
        ap=[ap.ap[0], [0, reps], [1, axis_len]],
    )


def _elu_exact(nc, pool, psum_ap, bias_ap, out_ap, shape, tag):
    """out = elu(psum + bias), bias per-partition AP [P,1]. 4 DVE + 1 ACT."""
    t = pool.tile(shape, F32, tag=f"{tag}_t", name=f"{tag}_t")
    nc.vector.tensor_scalar(t[:], psum_ap, bias_ap, None, ALU.add)
    mn = pool.tile(shape, F32, tag=f"{tag}_mn", name=f"{tag}_mn")
    nc.vector.tensor_scalar(mn[:], t[:], 0.0, None, ALU.min)
    ex = pool.tile(shape, F32, tag=f"{tag}_ex", name=f"{tag}_ex")
    nc.scalar.activation(ex[:], mn[:], ACTF.Exp)
    om = pool.tile(shape, F32, tag=f"{tag}_om", name=f"{tag}_om")
    nc.vector.scalar_tensor_tensor(om[:], t[:], 0.0, ex[:], ALU.max, ALU.add)
    nc.vector.tensor_scalar(out_ap, om[:], -1.0, None, ALU.add)


def _elu_shift(nc, pool, psum_ap, bias_ap, out_ap, shape, tag):
    """out = elu(psum + bias) + 1 = relu(t) + exp(min(t,0)). 3 DVE + 1 ACT."""
    t = pool.tile(shape, F32, tag=f"{tag}_t", name=f"{tag}_t")
    nc.vector.tensor_scalar(t[:], psum_ap, bias_ap, None, ALU.add)
    mn = pool.tile(shape, F32, tag=f"{tag}_mn", name=f"{tag}_mn")
    nc.vector.tensor_scalar(mn[:], t[:], 0.0, None, ALU.min)
    ex = pool.tile(shape, F32, tag=f"{tag}_ex", name=f"{tag}_ex")
    nc.scalar.activation(ex[:], mn[:], ACTF.Exp)
    nc.vector.scalar_tensor_tensor(out_ap, t[:], 0.0, ex[:], ALU.max, ALU.add)


def build():
    nc = bacc.Bacc(None)

    # ---------------- DRAM I/O ----------------
    act_d = nc.dram_tensor("act_t", [A, T, BL], F32, kind="ExternalInput")
    osc_d = nc.dram_tensor("osc_t", [64, T, BL], F32, kind="ExternalInput")
    mu_d = nc.dram_tensor("mu_t", [128, 2, BL], F32, kind="ExternalInput")
    mean_d = nc.dram_tensor("mean_t", [128, 2, BL], F32, kind="ExternalInput")
    mot_d = nc.dram_tensor("mot_t", [64, BL], F32, kind="ExternalInput")
    rob_d = nc.dram_tensor("rob_t", [64, BL], F32, kind="ExternalInput")
    st_d = nc.dram_tensor("st_t", [128, 4, BL], F32, kind="ExternalInput")

    gk_d = nc.dram_tensor("gk", [768, 1536], F32, kind="ExternalInput")
    rk_d = nc.dram_tensor("rk", [512, 1536], BF16, kind="ExternalInput")
    wmot_d = nc.dram_tensor("wmot", [64, 256], F32, kind="ExternalInput")
    wrob_d = nc.dram_tensor("wrob", [64, 256], F32, kind="ExternalInput")
    wcomb_d = nc.dram_tensor("wcomb", [1024, 512], F32, kind="ExternalInput")
    woscr_d = nc.dram_tensor("woscr", [64, 256], F32, kind="ExternalInput")
    # wosci padded to [256,256] then k-chunked [128, 2, 256] (top 64 rows zero)
    wosci_d = nc.dram_tensor("wosci", [128, 2, 256], F32, kind="ExternalInput")
    wout_d = nc.dram_tensor("wout", [512, 1], F32, kind="ExternalInput")

    bias_xp_d = nc.dram_tensor("bias_xp", [128, 12], F32, kind="ExternalInput")
    b1h_d = nc.dram_tensor("b1h", [128, 4], F32, kind="ExternalInput")
    boscr_d = nc.dram_tensor("boscr", [128, 2], F32, kind="ExternalInput")
    bosci_d = nc.dram_tensor("bosci", [128, 2], F32, kind="ExternalInput")
    bmot_d = nc.dram_tensor("bmot", [128, 2], F32, kind="ExternalInput")
    brob_d = nc.dram_tensor("brob", [128, 2], F32, kind="ExternalInput")
    bcomb_d = nc.dram_tensor("bcomb", [128, 4], F32, kind="ExternalInput")
    bout_d = nc.dram_tensor("bout", [1, 1], F32, kind="ExternalInput")

    outT_d = nc.dram_tensor("outT", [1, BL], F32, kind="ExternalOutput")
    hfin_d = nc.dram_tensor("hfin", [128, 4, BL], F32, kind="ExternalOutput")

    with tile.TileContext(nc) as tc:
        _build_body(
            nc, tc,
            act_d, osc_d, mu_d, mean_d, mot_d, rob_d, st_d,
            gk_d, rk_d, wmot_d, wrob_d, wcomb_d, woscr_d, wosci_d, wout_d,
            bias_xp_d, b1h_d, boscr_d, bosci_d, bmot_d, brob_d, bcomb_d, bout_d,
            outT_d, hfin_d,
        )
    nc.compile()
    return nc


def _build_body(
    nc, tc,
    act_d, osc_d, mu_d, mean_d, mot_d, rob_d, st_d,
    gk_d, rk_d, wmot_d, wrob_d, wcomb_d, woscr_d, wosci_d, wout_d,
    bias_xp_d, b1h_d, boscr_d, bosci_d, bmot_d, brob_d, bcomb_d, bout_d,
    outT_d, hfin_d,
):
    import contextlib

    ctx = contextlib.ExitStack()
    with ctx:
        cst = ctx.enter_context(tc.tile_pool(name="cst", bufs=1))
        xp_pool = ctx.enter_context(tc.tile_pool(name="xp_pool", bufs=2))
        in_pool = ctx.enter_context(tc.tile_pool(name="in_pool", bufs=2))
        tmp_pool = ctx.enter_context(tc.tile_pool(name="tmp_pool", bufs=3))
        sc_pool = ctx.enter_context(tc.tile_pool(name="sc_pool", bufs=3))
        ps_big = ctx.enter_context(tc.tile_pool(name="ps_big", bufs=3, space="PSUM"))
        ps_scan = ctx.enter_context(tc.tile_pool(name="ps_scan", bufs=2, space="PSUM"))
        ps_small = ctx.enter_context(tc.tile_pool(name="ps_small", bufs=1, space="PSUM"))

        dma = nc.gpsimd.dma_start

        # ---------------- persistent weight/const tiles ----------------
        gk_sb = cst.tile([128, 6, 12, 128], F32R)
        dma(out=gk_sb[:], in_=gk_d.rearrange("(kc p) (m j) -> p kc m j", p=128, j=128).bitcast(F32R))
        rk_sb = cst.tile([128, 4, 12, 128], BF16)
        dma(out=rk_sb[:], in_=rk_d.rearrange("(kc p) (m j) -> p kc m j", p=128, j=128))
        wcomb_sb = cst.tile([128, 8, 512], F32R)
        dma(out=wcomb_sb[:], in_=wcomb_d.rearrange("(kc p) n -> p kc n", p=128).bitcast(F32R))
        wmot_sb = cst.tile([64, 2, 128], F32R)
        dma(out=wmot_sb[:], in_=wmot_d.rearrange("k (m j) -> k m j", j=128).bitcast(F32R))
        wrob_sb = cst.tile([64, 2, 128], F32R)
        dma(out=wrob_sb[:], in_=wrob_d.rearrange("k (m j) -> k m j", j=128).bitcast(F32R))
        woscr_sb = cst.tile([64, 2, 128], F32R)
        dma(out=woscr_sb[:], in_=woscr_d.rearrange("k (m j) -> k m j", j=128).bitcast(F32R))
        wosci_sb = cst.tile([128, 2, 256], F32R)
        dma(out=wosci_sb[:], in_=wosci_d[:].bitcast(F32R))
        wout_sb = cst.tile([128, 4, 1], F32R)
        dma(out=wout_sb[:], in_=wout_d.rearrange("(kc p) n -> p kc n", p=128).bitcast(F32R))

        bias_xp = cst.tile([128, 12], F32)
        dma(out=bias_xp[:], in_=bias_xp_d[:])
        b1h = cst.tile([128, 4], F32)
        dma(out=b1h[:], in_=b1h_d[:])
        boscr = cst.tile([128, 2], F32)
        dma(out=boscr[:], in_=boscr_d[:])
        bosci = cst.tile([128, 2], F32)
        dma(out=bosci[:], in_=bosci_d[:])
        bmot = cst.tile([128, 2], F32)
        dma(out=bmot[:], in_=bmot_d[:])
        brob = cst.tile([128, 2], F32)
        dma(out=brob[:], in_=brob_d[:])
        bcomb = cst.tile([128, 4], F32)
        dma(out=bcomb[:], in_=bcomb_d[:])
        bout = cst.tile([1, 1], F32)
        dma(out=bout[:], in_=bout_d[:])

        mu_sb = cst.tile([128, 2, BL], F32)
        dma(out=mu_sb[:], in_=mu_d[:])
        mean_sb = cst.tile([128, 2, BL], F32)
        dma(out=mean_sb[:], in_=mean_d[:])
        mot_sb = cst.tile([64, BL], F32R)
        dma(out=mot_sb[:], in_=mot_d[:].bitcast(F32R))
        rob_sb = cst.tile([64, BL], F32R)
        dma(out=rob_sb[:], in_=rob_d[:].bitcast(F32R))
        st_sb = cst.tile([128, 4, BL], F32R)
        dma(out=st_sb[:], in_=st_d[:].bitcast(F32R))

        hmax = cst.tile([128, 4, BL], F32R)

        # ---------------- phase 0: h0 ----------------
        ms_sb = cst.tile([128, 2, BL], F32R)
        rs_sb = cst.tile([128, 2, BL], F32R)
        for (w_sb, x_sb, b_sb, o_sb, tg) in (
            (wmot_sb, mot_sb, bmot, ms_sb, "ms"),
            (wrob_sb, rob_sb, brob, rs_sb, "rs"),
        ):
            ps0 = ps_small.tile([128, 2, BL], F32, tag="ps0")
            for m in range(2):
                nc.tensor.matmul(ps0[:, m, :], w_sb[:, m, :], x_sb[:], start=True, stop=True)
            for m in range(2):
                _elu_exact(nc, tmp_pool, ps0[:, m, :], b_sb[:, m:m + 1],
                           o_sb[:, m, :], [128, BL], tg)

        h0ps = ps_small.tile([128, 4, BL], F32, tag="h0ps")
        rhs_h0 = [ms_sb[:, 0, :], ms_sb[:, 1, :], rs_sb[:, 0, :], rs_sb[:, 1, :],
                  st_sb[:, 0, :], st_sb[:, 1, :], st_sb[:, 2, :], st_sb[:, 3, :]]
        for m in range(4):
            for kc in range(8):
                nc.tensor.matmul(h0ps[:, m, :], wcomb_sb[:, kc, m * 128:(m + 1) * 128],
                                 rhs_h0[kc], start=(kc == 0), stop=(kc == 7))
        h_cur = sc_pool.tile([128, 4, BL], F32R, tag="h")
        for m in range(4):
            _elu_exact(nc, tmp_pool, h0ps[:, m, :], bcomb[:, m:m + 1],
                       h_cur[:, m, :].bitcast(F32), [128, BL], "h0")
        h_bf = sc_pool.tile([128, 4, BL], BF16, tag="hbf")
        nc.scalar.activation(h_bf[:], h_cur[:].bitcast(F32), ACTF.Copy)

        # ---------------- phase 1 (emits thunks) ----------------
        def phase1_thunks(cc):
            """Build xp for chunk cc into a fresh xp_pool tile; list of thunks."""
            th = []
            state = {}

            def dma_in():
                act_sb = in_pool.tile([128, 2, TCH, BL], F32R, tag="act")
                dma(out=act_sb[:],
                    in_=act_d.rearrange("(at p) t b -> p at t b", p=128)[:, :, cc * TCH:(cc + 1) * TCH, :].bitcast(F32R))
                osc_sb = in_pool.tile([64, TCH * BL], F32R, tag="osc")
                dma(out=osc_sb[:],
                    in_=osc_d[:, cc * TCH:(cc + 1) * TCH, :].rearrange("k t b -> k (t b)").bitcast(F32R))
                state["act"] = act_sb
                state["osc"] = osc_sb
            th.append(dma_in)

            def act_math(at):
                def f():
                    a = state["act"]
                    v = a[:, at, :, :]
                    nc.vector.tensor_tensor(v, v, _bcast(mu_sb[:, at, :], TCH, BL), ALU.mult)
                    nc.vector.tensor_tensor(v, v, _bcast(mean_sb[:, at, :], TCH, BL), ALU.add)
                return f
            th.append(act_math(0))
            th.append(act_math(1))

            R = TCH * BL  # 512 rows

            def inp2_mm(m2):
                def f():
                    p = ps_big.tile([128, R], F32, tag="p1")
                    nc.tensor.matmul(p[:], woscr_sb[:, m2, :], state["osc"][:], start=True, stop=True)
                    state[f"p2_{m2}"] = p
                return f

            def inp2_post(m2):
                def f():
                    i2 = state.setdefault("inp2", in_pool.tile([128, 2, R], F32R, tag="inp2", name="inp2"))
                    _elu_shift(nc, tmp_pool, state[f"p2_{m2}"][:], boscr[:, m2:m2 + 1],
                               i2[:, m2, :], [128, R], "i2")
                return f
            for m2 in range(2):
                th.append(inp2_mm(m2))
                th.append(inp2_post(m2))

            def inp3_mm(m3):
                def f():
                    i2 = state["inp2"]
                    p = ps_big.tile([128, R], F32, tag="p1")
                    nc.tensor.matmul(p[:], wosci_sb[:, 0, m3 * 128:(m3 + 1) * 128],
                                     i2[:, 0, :], start=True, stop=False)
                    nc.tensor.matmul(p[:], wosci_sb[:, 1, m3 * 128:(m3 + 1) * 128],
                                     i2[:, 1, :], start=False, stop=True)
                    state[f"p3_{m3}"] = p
                return f

            def inp3_post(m3):
                def f():
                    i3 = state.setdefault("inp3", in_pool.tile([128, 2, R], F32R, tag="inp3", name="inp3"))
                    _elu_shift(nc, tmp_pool, state[f"p3_{m3}"][:], bosci[:, m3:m3 + 1],
                               i3[:, m3, :], [128, R], "i3")
                return f
            for m3 in range(2):
                th.append(inp3_mm(m3))
                th.append(inp3_post(m3))

            def alloc_xp():
                state["xp"] = xp_pool.tile([128, 12, R], F32, tag="xp", name="xp")
            th.append(alloc_xp)

            def xp_mm(m):
                def f():
                    a, i2, i3 = state["act"], state["inp2"], state["inp3"]
                    rhs = [a[:, 0, :, :].rearrange("p t b -> p (t b)"),
                           a[:, 1, :, :].rearrange("p t b -> p (t b)"),
                           i2[:, 0, :], i2[:, 1, :], i3[:, 0, :], i3[:, 1, :]]
                    p = ps_big.tile([128, R], F32, tag="p1")
                    for kc in range(6):
                        nc.tensor.matmul(p[:], gk_sb[:, kc, m, :], rhs[kc],
                                         start=(kc == 0), stop=(kc == 5))
                    nc.vector.tensor_scalar(state["xp"][:, m, :], p[:],
                                            bias_xp[:, m:m + 1], None, ALU.add)
                return f
            for m in range(12):
                th.append(xp_mm(m))

            def finish():
                return state["xp"]
            return th, state

        # prologue: chunk 0 fully
        th0, st0 = phase1_thunks(0)
        for f in th0:
            f()
        xp_cur = st0["xp"]

        # ---------------- phase 2: scan ----------------
        nonloc = {"h": h_cur, "hbf": h_bf}

        def scan_step(xp_sb, tl, first=False):
            rec = ps_scan.tile([128, 12, BL], F32, tag="rec")
            hbf = nonloc["hbf"]
            for jg in range(12):
                for kc in range(4):
                    nc.tensor.matmul(rec[:, jg, :], rk_sb[:, kc, jg, :], hbf[:, kc, :],
                                     start=(kc == 0), stop=(kc == 3))
            xps = xp_sb[:, :, tl * BL:(tl + 1) * BL]
            az = sc_pool.tile([128, 4, BL], F32, tag="az")
            nc.vector.tensor_tensor(az[:], rec[:, 0:4, :], xps[:, 0:4, :], ALU.add)
            z = sc_pool.tile([128, 4, BL], F32, tag="z")
            nc.scalar.activation(z[:], az[:], ACTF.Sigmoid)
            ar = sc_pool.tile([128, 4, BL], F32, tag="ar")
            nc.vector.tensor_tensor(ar[:], rec[:, 4:8, :], xps[:, 4:8, :], ALU.add)
            r = sc_pool.tile([128, 4, BL], F32, tag="r")
            nc.scalar.activation(r[:], ar[:], ACTF.Sigmoid)
            u = sc_pool.tile([128, 4, BL], F32, tag="u")
            b1h_b = bass.AP(tensor=b1h[:].tensor, offset=b1h[:].offset,
                            ap=[b1h[:].ap[0], [1, 4], [0, BL]])
            nc.vector.tensor_tensor(u[:], rec[:, 8:12, :], b1h_b, ALU.add)
            t1 = sc_pool.tile([128, 4, BL], F32, tag="t1")
            nc.vector.tensor_tensor(t1[:], u[:], r[:], ALU.mult)
            t2 = sc_pool.tile([128, 4, BL], F32, tag="t2")
            nc.vector.tensor_tensor(t2[:], t1[:], xps[:, 8:12, :], ALU.add)
            hc = sc_pool.tile([128, 4, BL], F32, tag="hc")
            nc.scalar.activation(hc[:], t2[:], ACTF.Tanh)
            h_old = nonloc["h"]
            d = sc_pool.tile([128, 4, BL], F32, tag="d")
            nc.vector.tensor_tensor(d[:], h_old[:].bitcast(F32), hc[:], ALU.subtract)
            e = sc_pool.tile([128, 4, BL], F32, tag="e")
            nc.vector.tensor_tensor(e[:], z[:], d[:], ALU.mult)
            hn = sc_pool.tile([128, 4, BL], F32R, tag="h")
            nc.vector.tensor_tensor(hn[:].bitcast(F32), hc[:], e[:], ALU.add)
            hbf_n = sc_pool.tile([128, 4, BL], BF16, tag="hbf")
            nc.scalar.activation(hbf_n[:], hn[:].bitcast(F32), ACTF.Copy)
            if first:
                nc.vector.tensor_copy(hmax[:], hn[:].bitcast(F32))
            else:
                nc.vector.tensor_tensor(hmax[:], hmax[:],
                                        hn[:].bitcast(F32), ALU.max)
            nonloc["h"] = hn
            nonloc["hbf"] = hbf_n

        for cc in range(NCH):
            if cc + 1 < NCH:
                th, stn = phase1_thunks(cc + 1)
            else:
                th, stn = [], None
            quota = (len(th) + TCH - 1) // TCH if th else 0
            ti = 0
            for tl in range(TCH):
                scan_step(xp_cur, tl, first=(cc == 0 and tl == 0))
                for _ in range(quota):
                    if ti < len(th):
                        th[ti]()
                        ti += 1
            while ti < len(th):
                th[ti]()
                ti += 1
            if stn is not None:
                xp_cur = stn["xp"]

        # ---------------- phase 3: head ----------------
        ops = ps_small.tile([1, BL], F32, tag="ops")
        for kc in range(4):
            nc.tensor.matmul(ops[:], wout_sb[:, kc, :], hmax[:, kc, :],
                             start=(kc == 0), stop=(kc == 3))
        outT_sb = cst.tile([1, BL], F32)
        _elu_exact(nc, tmp_pool, ops[:], bout[:], outT_sb[:], [1, BL], "out")
        dma(out=outT_d[:], in_=outT_sb[:])
        dma(out=hfin_d[:], in_=nonloc["h"][:].bitcast(F32))


# ---------------- host side ----------------

def _preprocess(inputs):
    f32 = np.float32
    gru_k = np.asarray(inputs["gru_k"], f32)
    gru_rk = np.asarray(inputs["gru_rk"], f32)
    gru_b = np.asarray(inputs["gru_b"], f32)
    W_osci = np.asarray(inputs["W_osci"], f32)

    cs23 = gru_k[256:768].sum(axis=0)  # [1536] correction for inp2'/inp3' (+1 shift)
    b0, b1 = gru_b[0], gru_b[1]
    bias_xp = np.empty(1536, f32)
    bias_xp[0:512] = b0[0:512] + b1[0:512] - cs23[0:512]
    bias_xp[512:1024] = b0[512:1024] + b1[512:1024] - cs23[512:1024]
    bias_xp[1024:1536] = b0[1024:1536] - cs23[1024:1536]
    b1h = b1[1024:1536]

    bosci_adj = np.asarray(inputs["b_osci"], f32) - W_osci.sum(axis=0)

    # pad W_osci to 256 rows (zeros for inp2 features 0:64, which it doesn't consume)
    wosci_packed = np.zeros((128, 2, 256), f32)
    wosci_packed[64:128, 0, :] = W_osci[0:64]
    wosci_packed[:, 1, :] = W_osci[64:192]

    shared = {
        "gk": np.ascontiguousarray(gru_k),
        "rk": np.ascontiguousarray(gru_rk.astype(ml_dtypes.bfloat16)),
        "wmot": np.ascontiguousarray(np.asarray(inputs["W_mot"], f32)),
        "wrob": np.ascontiguousarray(np.asarray(inputs["W_rob"], f32)),
        "wcomb": np.ascontiguousarray(np.asarray(inputs["W_comb"], f32)),
        "woscr": np.ascontiguousarray(np.asarray(inputs["W_oscr"], f32)),
        "wosci": wosci_packed,
        "wout": np.ascontiguousarray(np.asarray(inputs["W_out"], f32)),
        "bias_xp": np.ascontiguousarray(bias_xp.reshape(12, 128).T),
        "b1h": np.ascontiguousarray(b1h.reshape(4, 128).T),
        "boscr": np.ascontiguousarray(np.asarray(inputs["b_oscr"], f32).reshape(2, 128).T),
        "bosci": np.ascontiguousarray(bosci_adj.reshape(2, 128).T),
        "bmot": np.ascontiguousarray(np.asarray(inputs["b_mot"], f32).reshape(2, 128).T),
        "brob": np.ascontiguousarray(np.asarray(inputs["b_rob"], f32).reshape(2, 128).T),
        "bcomb": np.ascontiguousarray(np.asarray(inputs["b_comb"], f32).reshape(4, 128).T),
        "bout": np.asarray(inputs["b_out"], f32).reshape(1, 1),
    }

    action = np.asarray(inputs["action"], f32)
    osc = np.asarray(inputs["osc"], f32)
    mu = np.asarray(inputs["mu"], f32)
    mean = np.asarray(inputs["mean"], f32)
    motion = np.asarray(inputs["motion_state"], f32)
    robot = np.asarray(inputs["robot_state"], f32)
    state = np.asarray(inputs["state"], f32)

    in_maps = []
    for c in range(NCORES):
        sl = slice(c * BL, (c + 1) * BL)
        m = dict(shared)
        m["act_t"] = np.ascontiguousarray(action[sl].transpose(2, 1, 0))
        m["osc_t"] = np.ascontiguousarray(osc[sl, :, :64].transpose(2, 1, 0))
        m["mu_t"] = np.ascontiguousarray(mu[sl].T.reshape(2, 128, BL).transpose(1, 0, 2))
        m["mean_t"] = np.ascontiguousarray(mean[sl].T.reshape(2, 128, BL).transpose(1, 0, 2))
        m["mot_t"] = np.ascontiguousarray(motion[sl].T)
        m["rob_t"] = np.ascontiguousarray(robot[sl].T)
        m["st_t"] = np.ascontiguousarray(state[sl].T.reshape(4, 128, BL).transpose(1, 0, 2))
        in_maps.append(m)
    return in_maps


def kernel(**inputs):
    if "nc" not in _CACHE:
        _CACHE["nc"] = build()
    nc = _CACHE["nc"]
    in_maps = _preprocess(inputs)
    res = run_bass_kernel_spmd(nc, in_maps, list(range(NCORES)),
                               trace=bool(os.environ.get("KERNEL_TRACE")))
    _CACHE["last_res"] = res
    out = np.empty((B, 1), np.float32)
    h_final = np.empty((B, UG), np.float32)
    for c in range(NCORES):
        sl = slice(c * BL, (c + 1) * BL)
        r = res.results[c]
        out[sl, 0] = r["outT"][0]
        # hfin[p, kc, b] -> h_final[b, kc*128+p]
        h_final[sl] = r["hfin"].transpose(1, 0, 2).reshape(UG, BL).T
    return out, h_final


if __name__ == "__main__":
    import time

    t0 = time.time()
    nc = build()
    print(f"build+compile: {time.time() - t0:.1f}s")


# revision 9
# speedup vs baseline: 1.0020x; 1.0020x over previous
"""Trainium2 Bass kernel for nn_CriticRDDPG (RDDPG critic forward).

Data-parallel over batch across 8 NeuronCores (16 rows/core). All activations
live in transposed "featT" layout [feature_partition, batch_free]; the GRU
scan runs with bf16 stationary recurrent weights (fast weight load) and the
moving h^T operand, producing gate pre-activations directly in a packed
[128, 12, 16] PSUM layout so gate math uses full partitions and no
transposes are needed anywhere.
"""

import os
import sys

sys.path.insert(0, "/opt/trn_rl_repo")

import numpy as np
import ml_dtypes

import concourse.bass as bass
import concourse.tile as tile
from concourse import bacc, mybir
from concourse.bass_utils import run_bass_kernel_spmd

F32 = mybir.dt.float32
F32R = mybir.dt.float32r
BF16 = mybir.dt.bfloat16
ALU = mybir.AluOpType
ACTF = mybir.ActivationFunctionType

B, T, A = 128, 512, 256
NCORES = 8
BL = B // NCORES  # 16 batch rows per core
TCH = 32  # time steps per chunk
NCH = int(os.environ.get("KERNEL_NCH", T // TCH))  # chunks (16 full)
TT = NCH * TCH  # total steps compiled
UG = 512  # U_GRU
NJG = 12  # 1536 / 128 gate-feature tiles
NKC = 4  # 512 / 128 hidden k-chunks

_CACHE = {}


def _bcast(ap, reps, axis_len):
    """0-stride broadcast of a [128, n] AP to [128, n, reps] (or [128, reps, n])."""
    return bass.AP(
        tensor=ap.tensor,
        offset=ap.offset,
        ap=[ap.ap[0], [0, reps], [1, axis_len]],
    )


def _elu_exact(nc, pool, psum_ap, bias_ap, out_ap, shape, tag):
    """out = elu(psum + bias), bias per-partition AP [P,1]. 4 DVE + 1 ACT."""
    t = pool.tile(shape, F32, tag=f"{tag}_t", name=f"{tag}_t")
    nc.vector.tensor_scalar(t[:], psum_ap, bias_ap, None, ALU.add)
    mn = pool.tile(shape, F32, tag=f"{tag}_mn", name=f"{tag}_mn")
    nc.vector.tensor_scalar(mn[:], t[:], 0.0, None, ALU.min)
    ex = pool.tile(shape, F32, tag=f"{tag}_ex", name=f"{tag}_ex")
    nc.scalar.activation(ex[:], mn[:], ACTF.Exp)
    om = pool.tile(shape, F32, tag=f"{tag}_om", name=f"{tag}_om")
    nc.vector.scalar_tensor_tensor(om[:], t[:], 0.0, ex[:], ALU.max, ALU.add)
    nc.vector.tensor_scalar(out_ap, om[:], -1.0, None, ALU.add)


def _elu_shift(nc, pool, psum_ap, bias_ap, out_ap, shape, tag):
    """out = elu(psum + bias) + 1 = relu(t) + exp(min(t,0)). 3 DVE + 1 ACT."""
    t = pool.tile(shape, F32, tag=f"{tag}_t", name=f"{tag}_t")
    nc.vector.tensor_scalar(t[:], psum_ap, bias_ap, None, ALU.add)
    mn = pool.tile(shape, F32, tag=f"{tag}_mn", name=f"{tag}_mn")
    nc.vector.tensor_scalar(mn[:], t[:], 0.0, None, ALU.min)
    ex = pool.tile(shape, F32, tag=f"{tag}_ex", name=f"{tag}_ex")
    nc.scalar.activation(ex[:], mn[:], ACTF.Exp)
    nc.vector.scalar_tensor_tensor(out_ap, t[:], 0.0, ex[:], ALU.max, ALU.add)


def build():
    nc = bacc.Bacc(None)

    # ---------------- DRAM I/O ----------------
    act_d = nc.dram_tensor("act_t", [A, T, BL], F32, kind="ExternalInput")
    osc_d = nc.dram_tensor("osc_t", [64, T, BL], F32, kind="ExternalInput")
    mu_d = nc.dram_tensor("mu_t", [128, 2, BL], F32, kind="ExternalInput")
    mean_d = nc.dram_tensor("mean_t", [128, 2, BL], F32, kind="ExternalInput")
    mot_d = nc.dram_tensor("mot_t", [64, BL], F32, kind="ExternalInput")
    rob_d = nc.dram_tensor("rob_t", [64, BL], F32, kind="ExternalInput")
    st_d = nc.dram_tensor("st_t", [128, 4, BL], F32, kind="ExternalInput")

    gk_d = nc.dram_tensor("gk", [768, 1536], F32, kind="ExternalInput")
    rk_d = nc.dram_tensor("rk", [512, 1536], BF16, kind="ExternalInput")
    wmot_d = nc.dram_tensor("wmot", [64, 256], F32, kind="ExternalInput")
    wrob_d = nc.dram_tensor("wrob", [64, 256], F32, kind="ExternalInput")
    wcomb_d = nc.dram_tensor("wcomb", [1024, 512], F32, kind="ExternalInput")
    woscr_d = nc.dram_tensor("woscr", [64, 256], F32, kind="ExternalInput")
    # wosci padded to [256,256] then k-chunked [128, 2, 256] (top 64 rows zero)
    wosci_d = nc.dram_tensor("wosci", [128, 2, 256], F32, kind="ExternalInput")
    wout_d = nc.dram_tensor("wout", [512, 1], F32, kind="ExternalInput")

    bias_xp_d = nc.dram_tensor("bias_xp", [128, 12], F32, kind="ExternalInput")
    b1h_d = nc.dram_tensor("b1h", [128, 4], F32, kind="ExternalInput")
    boscr_d = nc.dram_tensor("boscr", [128, 2], F32, kind="ExternalInput")
    bosci_d = nc.dram_tensor("bosci", [128, 2], F32, kind="ExternalInput")
    bmot_d = nc.dram_tensor("bmot", [128, 2], F32, kind="ExternalInput")
    brob_d = nc.dram_tensor("brob", [128, 2], F32, kind="ExternalInput")
    bcomb_d = nc.dram_tensor("bcomb", [128, 4], F32, kind="ExternalInput")
    bout_d = nc.dram_tensor("bout", [1, 1], F32, kind="ExternalInput")

    outT_d = nc.dram_tensor("outT", [1, BL], F32, kind="ExternalOutput")
    hfin_d = nc.dram_tensor("hfin", [128, 4, BL], F32, kind="ExternalOutput")

    with tile.TileContext(nc) as tc:
        _build_body(
            nc, tc,
            act_d, osc_d, mu_d, mean_d, mot_d, rob_d, st_d,
            gk_d, rk_d, wmot_d, wrob_d, wcomb_d, woscr_d, wosci_d, wout_d,
            bias_xp_d, b1h_d, boscr_d, bosci_d, bmot_d, brob_d, bcomb_d, bout_d,
            outT_d, hfin_d,
        )
    nc.compile()
    return nc


def _build_body(
    nc, tc,
    act_d, osc_d, mu_d, mean_d, mot_d, rob_d, st_d,
    gk_d, rk_d, wmot_d, wrob_d, wcomb_d, woscr_d, wosci_d, wout_d,
    bias_xp_d, b1h_d, boscr_d, bosci_d, bmot_d, brob_d, bcomb_d, bout_d,
    outT_d, hfin_d,
):
    import contextlib

    ctx = contextlib.ExitStack()
    with ctx:
        cst = ctx.enter_context(tc.tile_pool(name="cst", bufs=1))
        xp_pool = ctx.enter_context(tc.tile_pool(name="xp_pool", bufs=2))
        in_pool = ctx.enter_context(tc.tile_pool(name="in_pool", bufs=2))
        tmp_pool = ctx.enter_context(tc.tile_pool(name="tmp_pool", bufs=3))
        sc_pool = ctx.enter_context(tc.tile_pool(name="sc_pool", bufs=3))
        ps_big = ctx.enter_context(tc.tile_pool(name="ps_big", bufs=3, space="PSUM"))
        ps_scan = ctx.enter_context(tc.tile_pool(name="ps_scan", bufs=2, space="PSUM"))
        ps_small = ctx.enter_context(tc.tile_pool(name="ps_small", bufs=1, space="PSUM"))

        dma = nc.gpsimd.dma_start

        # ---------------- persistent weight/const tiles ----------------
        gk_sb = cst.tile([128, 6, 12, 128], F32R)
        dma(out=gk_sb[:], in_=gk_d.rearrange("(kc p) (m j) -> p kc m j", p=128, j=128).bitcast(F32R))
        rk_sb = cst.tile([128, 4, 12, 128], BF16)
        dma(out=rk_sb[:], in_=rk_d.rearrange("(kc p) (m j) -> p kc m j", p=128, j=128))
        wcomb_sb = cst.tile([128, 8, 512], F32R)
        dma(out=wcomb_sb[:], in_=wcomb_d.rearrange("(kc p) n -> p kc n", p=128).bitcast(F32R))
        wmot_sb = cst.tile([64, 2, 128], F32R)
        dma(out=wmot_sb[:], in_=wmot_d.rearrange("k (m j) -> k m j", j=128).bitcast(F32R))
        wrob_sb = cst.tile([64, 2, 128], F32R)
        dma(out=wrob_sb[:], in_=wrob_d.rearrange("k (m j) -> k m j", j=128).bitcast(F32R))
        woscr_sb = cst.tile([64, 2, 128], F32R)
        dma(out=woscr_sb[:], in_=woscr_d.rearrange("k (m j) -> k m j", j=128).bitcast(F32R))
        wosci_sb = cst.tile([128, 2, 256], F32R)
        dma(out=wosci_sb[:], in_=wosci_d[:].bitcast(F32R))
        wout_sb = cst.tile([128, 4, 1], F32R)
        dma(out=wout_sb[:], in_=wout_d.rearrange("(kc p) n -> p kc n", p=128).bitcast(F32R))

        bias_xp = cst.tile([128, 12], F32)
        dma(out=bias_xp[:], in_=bias_xp_d[:])
        b1h = cst.tile([128, 4], F32)
        dma(out=b1h[:], in_=b1h_d[:])
        boscr = cst.tile([128, 2], F32)
        dma(out=boscr[:], in_=boscr_d[:])
        bosci = cst.tile([128, 2], F32)
        dma(out=bosci[:], in_=bosci_d[:])
        bmot = cst.tile([128, 2], F32)
        dma(out=bmot[:], in_=bmot_d[:])
        brob = cst.tile([128, 2], F32)
        dma(out=brob[:], in_=brob_d[:])
        bcomb = cst.tile([128, 4], F32)
        dma(out=bcomb[:], in_=bcomb_d[:])
        bout = cst.tile([1, 1], F32)
        dma(out=bout[:], in_=bout_d[:])

        mu_sb = cst.tile([128, 2, BL], F32)
        dma(out=mu_sb[:], in_=mu_d[:])
        mean_sb = cst.tile([128, 2, BL], F32)
        dma(out=mean_sb[:], in_=mean_d[:])
        mot_sb = cst.tile([64, BL], F32R)
        dma(out=mot_sb[:], in_=mot_d[:].bitcast(F32R))
        rob_sb = cst.tile([64, BL], F32R)
        dma(out=rob_sb[:], in_=rob_d[:].bitcast(F32R))
        st_sb = cst.tile([128, 4, BL], F32R)
        dma(out=st_sb[:], in_=st_d[:].bitcast(F32R))

        hmax = cst.tile([128, 4, BL], F32R)

        # ---------------- phase 0: h0 ----------------
        ms_sb = cst.tile([128, 2, BL], F32R)
        rs_sb = cst.tile([128, 2, BL], F32R)
        for (w_sb, x_sb, b_sb, o_sb, tg) in (
            (wmot_sb, mot_sb, bmot, ms_sb, "ms"),
            (wrob_sb, rob_sb, brob, rs_sb, "rs"),
        ):
            ps0 = ps_small.tile([128, 2, BL], F32, tag="ps0")
            for m in range(2):
                nc.tensor.matmul(ps0[:, m, :], w_sb[:, m, :], x_sb[:], start=True, stop=True)
            for m in range(2):
                _elu_exact(nc, tmp_pool, ps0[:, m, :], b_sb[:, m:m + 1],
                           o_sb[:, m, :], [128, BL], tg)

        h0ps = ps_small.tile([128, 4, BL], F32, tag="h0ps")
        rhs_h0 = [ms_sb[:, 0, :], ms_sb[:, 1, :], rs_sb[:, 0, :], rs_sb[:, 1, :],
                  st_sb[:, 0, :], st_sb[:, 1, :], st_sb[:, 2, :], st_sb[:, 3, :]]
        for m in range(4):
            for kc in range(8):
                nc.tensor.matmul(h0ps[:, m, :], wcomb_sb[:, kc, m * 128:(m + 1) * 128],
                                 rhs_h0[kc], start=(kc == 0), stop=(kc == 7))
        h_cur = sc_pool.tile([128, 4, BL], F32R, tag="h")
        for m in range(4):
            _elu_exact(nc, tmp_pool, h0ps[:, m, :], bcomb[:, m:m + 1],
                       h_cur[:, m, :].bitcast(F32), [128, BL], "h0")
        h_bf = sc_pool.tile([128, 4, BL], BF16, tag="hbf")
        nc.scalar.activation(h_bf[:], h_cur[:].bitcast(F32), ACTF.Copy)

        # ---------------- phase 1 (emits thunks) ----------------
        def phase1_thunks(cc):
            """Build xp for chunk cc into a fresh xp_pool tile; list of thunks."""
            th = []
            state = {}

            def dma_in():
                act_sb = in_pool.tile([128, 2, TCH, BL], F32R, tag="act")
                dma(out=act_sb[:],
                    in_=act_d.rearrange("(at p) t b -> p at t b", p=128)[:, :, cc * TCH:(cc + 1) * TCH, :].bitcast(F32R))
                osc_sb = in_pool.tile([64, TCH * BL], F32R, tag="osc")
                dma(out=osc_sb[:],
                    in_=osc_d[:, cc * TCH:(cc + 1) * TCH, :].rearrange("k t b -> k (t b)").bitcast(F32R))
                state["act"] = act_sb
                state["osc"] = osc_sb
            th.append(dma_in)

            def act_math(at):
                def f():
                    a = state["act"]
                    v = a[:, at, :, :]
                    nc.vector.tensor_tensor(v, v, _bcast(mu_sb[:, at, :], TCH, BL), ALU.mult)
                    nc.vector.tensor_tensor(v, v, _bcast(mean_sb[:, at, :], TCH, BL), ALU.add)
                return f
            th.append(act_math(0))
            th.append(act_math(1))

            R = TCH * BL  # 512 rows

            def inp2_mm(m2):
                def f():
                    p = ps_big.tile([128, R], F32, tag="p1")
                    nc.tensor.matmul(p[:], woscr_sb[:, m2, :], state["osc"][:], start=True, stop=True)
                    state[f"p2_{m2}"] = p
                return f

            def inp2_post(m2):
                def f():
                    i2 = state.setdefault("inp2", in_pool.tile([128, 2, R], F32R, tag="inp2", name="inp2"))
                    _elu_shift(nc, tmp_pool, state[f"p2_{m2}"][:], boscr[:, m2:m2 + 1],
                               i2[:, m2, :], [128, R], "i2")
                return f
            for m2 in range(2):
                th.append(inp2_mm(m2))
                th.append(inp2_post(m2))

            def inp3_mm(m3):
                def f():
                    i2 = state["inp2"]
                    p = ps_big.tile([128, R], F32, tag="p1")
                    nc.tensor.matmul(p[:], wosci_sb[:, 0, m3 * 128:(m3 + 1) * 128],
                                     i2[:, 0, :], start=True, stop=False)
                    nc.tensor.matmul(p[:], wosci_sb[:, 1, m3 * 128:(m3 + 1) * 128],
                                     i2[:, 1, :], start=False, stop=True)
                    state[f"p3_{m3}"] = p
                return f

            def inp3_post(m3):
                def f():
                    i3 = state.setdefault("inp3", in_pool.tile([128, 2, R], F32R, tag="inp3", name="inp3"))
                    _elu_shift(nc, tmp_pool, state[f"p3_{m3}"][:], bosci[:, m3:m3 + 1],
                               i3[:, m3, :], [128, R], "i3")
                return f
            for m3 in range(2):
                th.append(inp3_mm(m3))
                th.append(inp3_post(m3))

            def alloc_xp():
                state["xp"] = xp_pool.tile([128, 12, R], F32, tag="xp", name="xp")
            th.append(alloc_xp)

            def xp_mm(m):
                def f():
                    a, i2, i3 = state["act"], state["inp2"], state["inp3"]
                    rhs = [a[:, 0, :, :].rearrange("p t b -> p (t b)"),
                           a[:, 1, :, :].rearrange("p t b -> p (t b)"),
                           i2[:, 0, :], i2[:, 1, :], i3[:, 0, :], i3[:, 1, :]]
                    p = ps_big.tile([128, R], F32, tag="p1")
                    for kc in range(6):
                        nc.tensor.matmul(p[:], gk_sb[:, kc, m, :], rhs[kc],
                                         start=(kc == 0), stop=(kc == 5))
                    nc.vector.tensor_scalar(state["xp"][:, m, :], p[:],
                                            bias_xp[:, m:m + 1], None, ALU.add)
                return f
            for m in range(12):
                th.append(xp_mm(m))

            def finish():
                return state["xp"]
            return th, state

        # prologue: chunk 0 fully
        th0, st0 = phase1_thunks(0)
        for f in th0:
            f()
        xp_cur = st0["xp"]

        # ---------------- phase 2: scan ----------------
        nonloc = {"h": h_cur, "hbf": h_bf}

        def scan_step(xp_sb, tl, first=False):
            rec = ps_scan.tile([128, 12, BL], F32, tag="rec")
            hbf = nonloc["hbf"]
            for jg in range(12):
                for kc in range(4):
                    nc.tensor.matmul(rec[:, jg, :], rk_sb[:, kc, jg, :], hbf[:, kc, :],
                                     start=(kc == 0), stop=(kc == 3))
            xps = xp_sb[:, :, tl * BL:(tl + 1) * BL]
            azr = sc_pool.tile([128, 8, BL], F32, tag="azr")
            nc.vector.tensor_tensor(azr[:], rec[:, 0:8, :], xps[:, 0:8, :], ALU.add)
            zr = sc_pool.tile([128, 8, BL], F32, tag="zr")
            nc.scalar.activation(zr[:], azr[:], ACTF.Sigmoid)
            z = zr[:, 0:4, :]
            r = zr[:, 4:8, :]
            u = sc_pool.tile([128, 4, BL], F32, tag="u")
            b1h_b = bass.AP(tensor=b1h[:].tensor, offset=b1h[:].offset,
                            ap=[b1h[:].ap[0], [1, 4], [0, BL]])
            nc.vector.tensor_tensor(u[:], rec[:, 8:12, :], b1h_b, ALU.add)
            t1 = sc_pool.tile([128, 4, BL], F32, tag="t1")
            nc.vector.tensor_tensor(t1[:], u[:], r, ALU.mult)
            t2 = sc_pool.tile([128, 4, BL], F32, tag="t2")
            nc.vector.tensor_tensor(t2[:], t1[:], xps[:, 8:12, :], ALU.add)
            hc = sc_pool.tile([128, 4, BL], F32, tag="hc")
            nc.scalar.activation(hc[:], t2[:], ACTF.Tanh)
            h_old = nonloc["h"]
            d = sc_pool.tile([128, 4, BL], F32, tag="d")
            nc.vector.tensor_tensor(d[:], h_old[:].bitcast(F32), hc[:], ALU.subtract)
            e = sc_pool.tile([128, 4, BL], F32, tag="e")
            nc.vector.tensor_tensor(e[:], z, d[:], ALU.mult)
            hn = sc_pool.tile([128, 4, BL], F32R, tag="h")
            nc.vector.tensor_tensor(hn[:].bitcast(F32), hc[:], e[:], ALU.add)
            hbf_n = sc_pool.tile([128, 4, BL], BF16, tag="hbf")
            nc.scalar.activation(hbf_n[:], hn[:].bitcast(F32), ACTF.Copy)
            if first:
                nc.vector.tensor_copy(hmax[:], hn[:].bitcast(F32))
            else:
                nc.vector.tensor_tensor(hmax[:], hmax[:],
                                        hn[:].bitcast(F32), ALU.max)
            nonloc["h"] = hn
            nonloc["hbf"] = hbf_n

        for cc in range(NCH):
            if cc + 1 < NCH:
                th, stn = phase1_thunks(cc + 1)
            else:
                th, stn = [], None
            quota = (len(th) + TCH - 1) // TCH if th else 0
            ti = 0
            for tl in range(TCH):
                scan_step(xp_cur, tl, first=(cc == 0 and tl == 0))
                for _ in range(quota):
                    if ti < len(th):
                        th[ti]()
                        ti += 1
            while ti < len(th):
                th[ti]()
                ti += 1
            if stn is not None:
                xp_cur = stn["xp"]

        # ---------------- phase 3: head ----------------
        ops = ps_small.tile([1, BL], F32, tag="ops")
        for kc in range(4):
            nc.tensor.matmul(ops[:], wout_sb[:, kc, :], hmax[:, kc, :],
                             start=(kc == 0), stop=(kc == 3))
        outT_sb = cst.tile([1, BL], F32)
        _elu_exact(nc, tmp_pool, ops[:], bout[:], outT_sb[:], [1, BL], "out")
        dma(out=outT_d[:], in_=outT_sb[:])
        dma(out=hfin_d[:], in_=nonloc["h"][:].bitcast(F32))


# ---------------- host side ----------------

def _preprocess(inputs):
    f32 = np.float32
    gru_k = np.asarray(inputs["gru_k"], f32)
    gru_rk = np.asarray(inputs["gru_rk"], f32)
    gru_b = np.asarray(inputs["gru_b"], f32)
    W_osci = np.asarray(inputs["W_osci"], f32)

    cs23 = gru_k[256:768].sum(axis=0)  # [1536] correction for inp2'/inp3' (+1 shift)
    b0, b1 = gru_b[0], gru_b[1]
    bias_xp = np.empty(1536, f32)
    bias_xp[0:512] = b0[0:512] + b1[0:512] - cs23[0:512]
    bias_xp[512:1024] = b0[512:1024] + b1[512:1024] - cs23[512:1024]
    bias_xp[1024:1536] = b0[1024:1536] - cs23[1024:1536]
    b1h = b1[1024:1536]

    bosci_adj = np.asarray(inputs["b_osci"], f32) - W_osci.sum(axis=0)

    # pad W_osci to 256 rows (zeros for inp2 features 0:64, which it doesn't consume)
    wosci_packed = np.zeros((128, 2, 256), f32)
    wosci_packed[64:128, 0, :] = W_osci[0:64]
    wosci_packed[:, 1, :] = W_osci[64:192]

    shared = {
        "gk": np.ascontiguousarray(gru_k),
        "rk": np.ascontiguousarray(gru_rk.astype(ml_dtypes.bfloat16)),
        "wmot": np.ascontiguousarray(np.asarray(inputs["W_mot"], f32)),
        "wrob": np.ascontiguousarray(np.asarray(inputs["W_rob"], f32)),
        "wcomb": np.ascontiguousarray(np.asarray(inputs["W_comb"], f32)),
        "woscr": np.ascontiguousarray(np.asarray(inputs["W_oscr"], f32)),
        "wosci": wosci_packed,
        "wout": np.ascontiguousarray(np.asarray(inputs["W_out"], f32)),
        "bias_xp": np.ascontiguousarray(bias_xp.reshape(12, 128).T),
        "b1h": np.ascontiguousarray(b1h.reshape(4, 128).T),
        "boscr": np.ascontiguousarray(np.asarray(inputs["b_oscr"], f32).reshape(2, 128).T),
        "bosci": np.ascontiguousarray(bosci_adj.reshape(2, 128).T),
        "bmot": np.ascontiguousarray(np.asarray(inputs["b_mot"], f32).reshape(2, 128).T),
        "brob": np.ascontiguousarray(np.asarray(inputs["b_rob"], f32).reshape(2, 128).T),
        "bcomb": np.ascontiguousarray(np.asarray(inputs["b_comb"], f32).reshape(4, 128).T),
        "bout": np.asarray(inputs["b_out"], f32).reshape(1, 1),
    }

    action = np.asarray(inputs["action"], f32)
    osc = np.asarray(inputs["osc"], f32)
    mu = np.asarray(inputs["mu"], f32)
    mean = np.asarray(inputs["mean"], f32)
    motion = np.asarray(inputs["motion_state"], f32)
    robot = np.asarray(inputs["robot_state"], f32)
    state = np.asarray(inputs["state"], f32)

    in_maps = []
    for c in range(NCORES):
        sl = slice(c * BL, (c + 1) * BL)
        m = dict(shared)
        m["act_t"] = np.ascontiguousarray(action[sl].transpose(2, 1, 0))
        m["osc_t"] = np.ascontiguousarray(osc[sl, :, :64].transpose(2, 1, 0))
        m["mu_t"] = np.ascontiguousarray(mu[sl].T.reshape(2, 128, BL).transpose(1, 0, 2))
        m["mean_t"] = np.ascontiguousarray(mean[sl].T.reshape(2, 128, BL).transpose(1, 0, 2))
        m["mot_t"] = np.ascontiguousarray(motion[sl].T)
        m["rob_t"] = np.ascontiguousarray(robot[sl].T)
        m["st_t"] = np.ascontiguousarray(state[sl].T.reshape(4, 128, BL).transpose(1, 0, 2))
        in_maps.append(m)
    return in_maps


def kernel(**inputs):
    if "nc" not in _CACHE:
        _CACHE["nc"] = build()
    nc = _CACHE["nc"]
    in_maps = _preprocess(inputs)
    res = run_bass_kernel_spmd(nc, in_maps, list(range(NCORES)),
                               trace=bool(os.environ.get("KERNEL_TRACE")))
    _CACHE["last_res"] = res
    out = np.empty((B, 1), np.float32)
    h_final = np.empty((B, UG), np.float32)
    for c in range(NCORES):
        sl = slice(c * BL, (c + 1) * BL)
        r = res.results[c]
        out[sl, 0] = r["outT"][0]
        # hfin[p, kc, b] -> h_final[b, kc*128+p]
        h_final[sl] = r["hfin"].transpose(1, 0, 2).reshape(UG, BL).T
    return out, h_final


if __name__ == "__main__":
    import time

    t0 = time.time()
    nc = build()
    print(f"build+compile: {time.time() - t0:.1f}s")
